# revision 13
# baseline (speedup 1.0000x reference)
"""Trainium2 Bass kernel for GQA attention prefill with KV cache.

Problem: B=2, S=1024, DIM=4096, 32 q-heads / 8 kv-heads (GQA 4:1), HEAD_DIM=128,
CACHE_LEN=1024 (KV_LEN=2048), RoPE on q/k, causal mask over the new block.

Sharding: tensor-parallel over heads across 8 cores. Each core gets 4 q-heads /
1 kv-head: wq/wk/wv column shards, wo row shard, KV-cache shard. Each core
computes a full-shape partial output (row-parallel wo); host sums the partials.

Layout strategy (per core, all matmuls bf16 with fp32 PSUM accumulation):
  - x is pre-transposed on host to xT [DIM, B*S]; projections produce qT/kT
    [head_dim, s] directly (head_dim on partitions).
  - head_dim of q/k is permuted (even indices then odd) so RoPE's complex
    rotation acts on partition halves [0:64) re / [64:128) im. wq/wk columns
    and cache_k are permuted identically on host (scores are invariant).
  - Attention runs over HEAD PAIRS sharing each kv-chunk stationary (GQA):
    per (sb, head-pair, mc) one [128,2,512] PSUM tile holds both heads'
    transposed scores; ONE ScalarE exp (N up to 1024) covers both; the two
    AV matmuls reuse the v-chunk stationary. Causal handled by computing only
    valid (chunk, s-block) pairs + a triangular additive tile on diagonal
    chunks.
  - softmax denominator: VectorE accumulates exp chunks into acc [128,2,512];
    two ones-stationary matmuls reduce partitions into a [1,2,512] PSUM tile
    (sharing the scores pool slots), reciprocal_approx_fast on VectorE, a K=1
    ones matmul broadcasts 1/denom across partitions, VectorE normalizes while
    evacuating AV PSUM into outT.
  - O-proj: out[s, n] = sum_h outT_h.T @ wo_h, partial f32 written to DRAM.
  - PSUM budget: mm pool (proj/oproj accum + attn AV) 4 banks, scores pool
    (scores/den/rb) 2x2 banks = exactly 8.
"""
import math
import numpy as np
import ml_dtypes

import concourse.bacc as bacc
import concourse.mybir as mybir
import concourse.tile as tile
from concourse.bass_utils import run_bass_kernel_spmd

BF16 = ml_dtypes.bfloat16
F32 = mybir.dt.float32
BF = mybir.dt.bfloat16

B, S, DIM = 2, 1024, 4096
N_HEADS, N_KV = 32, 8
HD = 128
QH = N_HEADS // 8          # q heads per core = 4
CACHE = 1024
KV = CACHE + S             # 2048
KCH = DIM // 128           # 32 contraction chunks
NCH = KV // 128            # 16 kv chunks
SBW = 512                  # s-block width
NSB = S // SBW             # 2
SCALE = 1.0 / math.sqrt(HD)

_CACHE = {}


def _pairs_for_mode(mode):
    """Per s-block list of (mc, n_off, tri, masked). First pair of each block
    must be full-width (n_off == 0)."""
    pairs = {sb: [] for sb in range(NSB)}
    for sb in range(NSB):
        sb_start = sb * SBW
        for mc in range(NCH):
            if mode == "causal" and mc >= CACHE // 128:
                s0 = (mc - CACHE // 128) * 128   # first query that sees this chunk
                if s0 >= sb_start + SBW:
                    continue                      # chunk entirely in the future
                n_off = max(0, s0 - sb_start)
                tri = s0 >= sb_start              # diagonal strip inside block
                pairs[sb].append((mc, n_off, tri, mode == "generic"))
            else:
                pairs[sb].append((mc, 0, False, mode == "generic"))
        # ensure first pair full width
        assert pairs[sb][0][1] == 0
    return pairs


def build_nc(mode, reps=1, phases=('proj', 'attn', 'oproj')):
    nc = bacc.Bacc("TRN2", target_bir_lowering=False, debug=False, num_devices=8)

    # xt pre-split on host into DRAM-contiguous pieces [b*8 + g*2 + hh]
    # so each startup DMA is a dense 8KB-per-partition burst.
    xt = nc.dram_tensor("xt", [B * 8, 128, 16, 256], mybir.dt.bfloat16, kind="ExternalInput")
    wq = nc.dram_tensor("wq", [128, QH, KCH, HD], mybir.dt.bfloat16, kind="ExternalInput")
    wk = nc.dram_tensor("wk", [128, KCH, HD], mybir.dt.bfloat16, kind="ExternalInput")
    wv = nc.dram_tensor("wv", [128, KCH, HD], mybir.dt.bfloat16, kind="ExternalInput")
    wo = nc.dram_tensor("wo", [128, QH, DIM], mybir.dt.bfloat16, kind="ExternalInput")
    kc = nc.dram_tensor("kc", [B, 128, CACHE], mybir.dt.bfloat16, kind="ExternalInput")
    vc = nc.dram_tensor("vc", [B, 128, CACHE], mybir.dt.bfloat16, kind="ExternalInput")
    cc = nc.dram_tensor("cc", [128, S], F32, kind="ExternalInput")
    ss = nc.dram_tensor("ss", [128, S], F32, kind="ExternalInput")
    tri = nc.dram_tensor("tri", [128, 128], F32, kind="ExternalInput")
    if mode == "generic":
        mt = nc.dram_tensor("maskt", [128, NCH, S], mybir.dt.bfloat16, kind="ExternalInput")
    out = nc.dram_tensor("o", [B * S, DIM], mybir.dt.bfloat16, kind="ExternalOutput")

    pairs = _pairs_for_mode(mode)

    with tile.TileContext(nc) as tc:
        with (
            tc.tile_pool(name="const", bufs=1) as const,
            tc.tile_pool(name="xpool", bufs=1) as xpool,
            tc.tile_pool(name="perb", bufs=2) as perb,
            tc.tile_pool(name="work", bufs=2) as work,
            tc.tile_pool(name="ps_mm", bufs=4, space="PSUM") as ps_mm,
            tc.tile_pool(name="ps_sc", bufs=2, space="PSUM") as ps_sc,
        ):
          from contextlib import nullcontext
          hints = (mybir.EngineType.PE, mybir.EngineType.DVE,
                   mybir.EngineType.Activation, mybir.EngineType.SP)
          with (tc.For_i(0, reps, 1, hint_engines=hints)
                if reps > 1 else nullcontext()):
            # ---- constants (emission order = DMA priority order) ----
            wk_sb = const.tile([128, KCH, HD], BF)
            nc.sync.dma_start(wk_sb, wk.ap())
            cc_sb = const.tile([128, S], F32)
            nc.sync.dma_start(cc_sb, cc.ap())
            ss_sb = const.tile([128, S], F32)
            nc.sync.dma_start(ss_sb, ss.ap())
            tri_sb = const.tile([128, 128], F32)
            nc.sync.dma_start(tri_sb, tri.ap())
            ones_sq = const.tile([128, 128], BF)
            nc.vector.memset(ones_sq, 1.0)
            wv_sb = const.tile([128, KCH, HD], BF)

            def rope_store(ps, dest, scol):
                """dest[0:64] = ps[0:64]*cc - ps[64:]*ss ; dest[64:] = ps[0:64]*ss + ps[64:]*cc
                ps: [128, SBW] psum f32; dest: [128, SBW] sbuf bf16 slice; scol: s offset.
                Every tensor_tensor keeps equal input base partitions (walrus
                birverifier requirement); cross terms write to the other half."""
                cslc = cc_sb[:, scol:scol + SBW]
                sslc = ss_sb[:, scol:scol + SBW]
                t1 = work.tile([128, SBW], F32, name="rope_t1")
                # t1[0:64] = re*cos ; t1[64:] = im*cos
                nc.vector.tensor_mul(t1, ps, cslc)
                # t2[0:64] = im*sin (from inputs @64) ; t2[64:] = re*sin (from @0)
                t2 = work.tile([128, SBW], F32, name="rope_t2")
                nc.vector.tensor_mul(t2[0:64, :], ps[64:128, :], sslc[64:128, :])
                nc.vector.tensor_mul(t2[64:128, :], ps[0:64, :], sslc[0:64, :])
                nc.vector.tensor_sub(dest[0:64, :], t1[0:64, :], t2[0:64, :])
                nc.vector.tensor_add(dest[64:128, :], t2[64:128, :], t1[64:128, :])

            for b in range(B):
                # ---- load x (transposed): pieces ordered so K-proj sb0 can
                # start as soon as its ko-chunks land ----
                # g-major piece order: the s-range needed first arrives first;
                # pieces alternate between the two HWDGE queues (sync/scalar).
                xT = xpool.tile([128, KCH, S], BF, name="xT")
                for pi_, (g, hh) in enumerate(
                        [(g, hh) for g in range(4) for hh in range(2)]):
                    eng = nc.sync if pi_ % 2 == 0 else nc.scalar
                    eng.dma_start(
                        xT[:, hh * 16:(hh + 1) * 16, g * 256:(g + 1) * 256],
                        xt.ap()[b * 8 + g * 2 + hh])
                if b == 0:
                    nc.sync.dma_start(wv_sb, wv.ap())
                kT = perb.tile([128, KV], BF, name="kT")
                nc.scalar.dma_start(kT[:, 0:CACHE], kc.ap()[b])
                v_sb = perb.tile([128, NCH * 128], BF, name="v_sb")
                nc.scalar.dma_start(v_sb[:, 0:CACHE], vc.ap()[b])

                # ---- K projection + RoPE ----
                for sb in (range(NSB) if 'proj' in phases else []):
                    ps = ps_mm.tile([128, SBW], F32, name="mm_ps")
                    for ko in range(KCH):
                        nc.tensor.matmul(ps, wk_sb[:, ko, :],
                                         xT[:, ko, sb * SBW:(sb + 1) * SBW],
                                         start=(ko == 0), stop=(ko == KCH - 1))
                    rope_store(ps, kT[:, CACHE + sb * SBW: CACHE + (sb + 1) * SBW],
                               sb * SBW)

                # ---- Q projection + RoPE ----
                qT = perb.tile([128, QH, S], BF, name="qT")
                for h in (range(QH) if 'proj' in phases else []):
                    wq_blk = work.tile([128, KCH, HD], BF, name="wq_blk")
                    nc.sync.dma_start(wq_blk, wq.ap()[:, h])
                    for sb in range(NSB):
                        ps = ps_mm.tile([128, SBW], F32, name="mm_ps")
                        for ko in range(KCH):
                            nc.tensor.matmul(ps, wq_blk[:, ko, :],
                                             xT[:, ko, sb * SBW:(sb + 1) * SBW],
                                             start=(ko == 0), stop=(ko == KCH - 1))
                        rope_store(ps, qT[:, h, sb * SBW:(sb + 1) * SBW], sb * SBW)

                # ---- V projection (natural [m, d] layout) ----
                for mt_i in (range(S // 128) if 'proj' in phases else []):
                    ps = ps_mm.tile([128, SBW], F32, name="mm_ps")
                    for ko in range(KCH):
                        nc.tensor.matmul(
                            ps[:, 0:128],
                            xT[:, ko, mt_i * 128:(mt_i + 1) * 128],
                            wv_sb[:, ko, :],
                            start=(ko == 0), stop=(ko == KCH - 1))
                    vcol = CACHE + mt_i * 128
                    if mt_i % 2 == 0:
                        nc.scalar.copy(v_sb[:, vcol:vcol + 128], ps[:, 0:128])
                    else:
                        nc.vector.tensor_copy(v_sb[:, vcol:vcol + 128], ps[:, 0:128])

                # ---- attention over head pairs sharing kv-chunk stationaries ----
                outT = None
                if 'attn' in phases or 'oproj' in phases:
                    outT = perb.tile([128, QH, S], BF, name="outT", bufs=1)
                    if 'attn' not in phases:
                        nc.vector.memset(outT, 0.0)
                for sb in (range(NSB) if 'attn' in phases else []):
                    plist = pairs[sb]
                    for hp in range(QH // 2):
                        h0 = 2 * hp
                        av0 = ps_mm.tile([128, SBW], F32, name="mm_ps")
                        av1 = ps_mm.tile([128, SBW], F32, name="mm_ps")
                        # two parallel accumulation chains (even/odd pairs)
                        # halve the serial DVE depth that gates exp via the
                        # ex-slot WAR.
                        accs = (work.tile([128, 2, SBW], BF, name="accA", bufs=1),
                                work.tile([128, 2, SBW], BF, name="accB", bufs=1))
                        for pi, (mc, n_off, use_tri, use_mask) in enumerate(plist):
                            w = SBW - n_off
                            sc = ps_sc.tile([128, 2, SBW], F32, name="sc_ps")
                            nc.tensor.matmul(
                                sc[:, 0, n_off:], kT[:, mc * 128:(mc + 1) * 128],
                                qT[:, h0, sb * SBW + n_off:(sb + 1) * SBW],
                                start=True, stop=True)
                            nc.tensor.matmul(
                                sc[:, 1, n_off:], kT[:, mc * 128:(mc + 1) * 128],
                                qT[:, h0 + 1, sb * SBW + n_off:(sb + 1) * SBW],
                                start=True, stop=True)
                            if use_tri:
                                nc.vector.tensor_add(
                                    sc[:, 0, n_off:n_off + 128],
                                    sc[:, 0, n_off:n_off + 128], tri_sb)
                                nc.vector.tensor_add(
                                    sc[:, 1, n_off:n_off + 128],
                                    sc[:, 1, n_off:n_off + 128], tri_sb)
                            if use_mask:
                                mblk = work.tile([128, SBW], BF, name="mblk", bufs=3)
                                nc.sync.dma_start(
                                    mblk, mt.ap()[:, mc, sb * SBW:(sb + 1) * SBW])
                                nc.vector.tensor_add(sc[:, 0, n_off:],
                                                     sc[:, 0, n_off:],
                                                     mblk[:, n_off:])
                                nc.vector.tensor_add(sc[:, 1, n_off:],
                                                     sc[:, 1, n_off:],
                                                     mblk[:, n_off:])
                            ex = work.tile([128, 2, SBW], BF, name="ex", bufs=6)
                            nc.scalar.activation(ex[:, :, n_off:], sc[:, :, n_off:],
                                                 mybir.ActivationFunctionType.Exp,
                                                 scale=SCALE)
                            nc.tensor.matmul(av0[:, n_off:],
                                             v_sb[:, mc * 128:(mc + 1) * 128],
                                             ex[:, 0, n_off:],
                                             start=(pi == 0), stop=(pi == len(plist) - 1))
                            nc.tensor.matmul(av1[:, n_off:],
                                             v_sb[:, mc * 128:(mc + 1) * 128],
                                             ex[:, 1, n_off:],
                                             start=(pi == 0), stop=(pi == len(plist) - 1))
                            acc = accs[pi % 2]
                            if pi < 2:
                                nc.vector.tensor_copy(acc, ex)
                            else:
                                nc.vector.tensor_add(acc[:, :, n_off:],
                                                     acc[:, :, n_off:],
                                                     ex[:, :, n_off:])
                        # denominator, pre-broadcast across partitions by an
                        # all-ones stationary: den_b[j, s] = sum_k accA + accB.
                        # TensorE never waits on the reciprocal chain.
                        den = ps_sc.tile([128, 2, SBW], F32, name="sc_ps")
                        for h in range(2):
                            nc.tensor.matmul(den[:, h, :], ones_sq, accs[0][:, h, :],
                                             start=True, stop=False)
                            nc.tensor.matmul(den[:, h, :], ones_sq, accs[1][:, h, :],
                                             start=False, stop=True)
                        rcf = work.tile([128, 2, SBW], F32, name="rcf")
                        nc.vector.reciprocal_approx_fast(rcf, den)
                        nc.vector.tensor_mul(outT[:, h0, sb * SBW:(sb + 1) * SBW],
                                             av0, rcf[:, 0, :])
                        nc.vector.tensor_mul(outT[:, h0 + 1, sb * SBW:(sb + 1) * SBW],
                                             av1, rcf[:, 1, :])

                # ---- O projection (partial, bf16) ----
                for nb in (range(DIM // SBW) if 'oproj' in phases else []):
                    wo_blk = work.tile([128, QH, SBW], BF, name="wo_blk", bufs=3)
                    nc.sync.dma_start(wo_blk[:, 0:2], wo.ap()[:, 0:2, nb * SBW:(nb + 1) * SBW])
                    nc.sync.dma_start(wo_blk[:, 2:4], wo.ap()[:, 2:4, nb * SBW:(nb + 1) * SBW])
                    for st in range(S // 128):
                        ps = ps_mm.tile([128, SBW], F32, name="mm_ps")
                        for h in range(QH):
                            nc.tensor.matmul(ps, outT[:, h, st * 128:(st + 1) * 128],
                                             wo_blk[:, h, :],
                                             start=(h == 0), stop=(h == QH - 1))
                        # DVE-only evacuation: ScalarE's queue may be busy
                        # with HWDGE transfers (next batch's xT pieces).
                        os_t = work.tile([128, SBW], BF, name="os_t", bufs=6)
                        nc.vector.tensor_copy(os_t, ps)
                        nc.sync.dma_start(
                            out.ap()[b * S + st * 128:b * S + (st + 1) * 128,
                                     nb * SBW:(nb + 1) * SBW], os_t)

    nc.compile()
    return nc


def _detect_mode(mask):
    m = np.asarray(mask).reshape(S, KV)
    if not np.all(m[:, :CACHE] == 0):
        return "generic"
    new = m[:, CACHE:]
    neg = np.min(new)
    if neg == 0:
        return "full" if np.all(new == 0) else "generic"
    causal = np.triu(np.full((S, S), neg, dtype=np.float32), k=1)
    if np.array_equal(new, causal):
        return "causal"
    return "generic"


def prepare_inputs(x, freqs_cos, freqs_sin, mask, cache_k, cache_v, wq, wk, wv, wo,
                   mode):
    """Build the 8 per-core in_maps (numpy, host-side sharding + layout)."""
    perm = np.concatenate([np.arange(0, HD, 2), np.arange(1, HD, 2)])
    x2 = np.asarray(x, np.float32).reshape(B * S, DIM)
    xt_full = x2.reshape(B * S, KCH, 128).transpose(2, 1, 0)  # [128,KCH,BS]
    # split into DRAM-contiguous pieces [b*8 + g*2 + hh] -> [128, 16, 256]
    xt = np.ascontiguousarray(
        xt_full.reshape(128, 2, 16, B, 4, 256)        # [p, hh, ko16, b, g, s256]
        .transpose(3, 4, 1, 0, 2, 5)                  # [b, g, hh, p, ko16, s256]
        .reshape(B * 8, 128, 16, 256)).astype(BF16)
    cos = np.asarray(freqs_cos, np.float32)
    sin = np.asarray(freqs_sin, np.float32)
    cc = np.ascontiguousarray(np.vstack([cos.T, cos.T]))  # [128, S]
    ss = np.ascontiguousarray(np.vstack([sin.T, sin.T]))
    m0 = np.asarray(mask, np.float32).reshape(S, KV)
    inv_scale = 1.0 / SCALE
    if mode == "causal":
        tri_np = np.ascontiguousarray(m0[0:128, CACHE:CACHE + 128].T) * inv_scale
    else:
        tri_np = np.zeros((128, 128), np.float32)
    maskt = None
    if mode == "generic":
        maskt = np.ascontiguousarray(
            (m0.T * inv_scale).reshape(NCH, 128, S).transpose(1, 0, 2)).astype(BF16)

    wq_f = np.asarray(wq, np.float32)
    wk_f = np.asarray(wk, np.float32)
    wv_f = np.asarray(wv, np.float32)
    wo_f = np.asarray(wo, np.float32)
    ck = np.asarray(cache_k, np.float32)
    cv = np.asarray(cache_v, np.float32)

    in_maps = []
    for c in range(8):
        wq_c = wq_f[:, c * 512:(c + 1) * 512].reshape(DIM, QH, HD)[:, :, perm]
        # [128, QH, KCH, HD]: [ki, h, ko, d] = wq_c[ko*128+ki, h, d]
        wq_r = np.ascontiguousarray(
            wq_c.reshape(KCH, 128, QH, HD).transpose(1, 2, 0, 3)).astype(BF16)
        wk_c = wk_f[:, c * 128:(c + 1) * 128][:, perm]
        wk_r = np.ascontiguousarray(
            wk_c.reshape(KCH, 128, HD).transpose(1, 0, 2)).astype(BF16)
        wv_c = wv_f[:, c * 128:(c + 1) * 128]
        wv_r = np.ascontiguousarray(
            wv_c.reshape(KCH, 128, HD).transpose(1, 0, 2)).astype(BF16)
        wo_c = wo_f[c * 512:(c + 1) * 512, :]
        wo_r = np.ascontiguousarray(
            wo_c.reshape(QH, 128, DIM).transpose(1, 0, 2)).astype(BF16)
        kc_r = np.ascontiguousarray(
            ck[:, c][:, :, perm].transpose(0, 2, 1)).astype(BF16)  # [B,128,CACHE]
        vc_r = np.ascontiguousarray(
            cv[:, c].reshape(B, CACHE // 128, 128, HD).transpose(0, 2, 1, 3)
            .reshape(B, 128, CACHE)).astype(BF16)
        im = {"xt": xt, "wq": wq_r, "wk": wk_r, "wv": wv_r, "wo": wo_r,
              "kc": kc_r, "vc": vc_r, "cc": cc, "ss": ss, "tri": tri_np}
        if maskt is not None:
            im["maskt"] = maskt
        in_maps.append(im)
    return in_maps


def kernel(x, freqs_cos, freqs_sin, mask, cache_k, cache_v, wq, wk, wv, wo):
    mode = _detect_mode(mask)
    in_maps = prepare_inputs(x, freqs_cos, freqs_sin, mask, cache_k, cache_v,
                             wq, wk, wv, wo, mode)
    if mode not in _CACHE:
        _CACHE[mode] = build_nc(mode)
    nc = _CACHE[mode]
    res = run_bass_kernel_spmd(nc, in_maps, core_ids=list(range(8)))
    total = res.results[0]["o"].astype(np.float32)
    for c in range(1, 8):
        total += res.results[c]["o"].astype(np.float32)
    return total.reshape(B, S, DIM)
